# revision 20
# baseline (speedup 1.0000x reference)
"""Causal multi-head attention on 8 TRN2 NeuronCores.

Problem: B=2, L=2048, H=16, E=64 (f32 in/out). B*H = 32 (batch, head)
slices are data-parallel: 4 slices per core, no cross-core comm.

v3 design: the exp over the causal half of S is the bottleneck engine
work (~58us/core on ScalarE alone). Split it across two engines and
double-buffer everything:

  - Global "round" pipeline over all 544 = 4*136 causal 128x128 blocks:
    each round is 12 blocks; 8 are exp'd by ScalarE (ACTIVATE FD=1024
    from a double-buffered 2-bank PSUM tile) and 4 by VectorE using a
    Schraudolph exp2 bit-trick (round(x*a+b) as int16 == bf16 bits of
    exp(x*SCALE), max rel err ~3.5%, validated ~8e-3 end-to-end) from a
    double-buffered 1-bank tile. PE fills round r+1 while consumers
    drain round r -- no engine ping-pong.
  - O chains are emitted one round after their trigger so their
    LDWEIGHTS never head-of-line-block the PE queue behind a pending
    consumer.
  - S^T[m, l] = K^T(dup) Q^T(dup) per block (contraction 128 = 2x dup of
    e=64 -> computes 2S, absorbed in exp scale; keeps PE at full clock).
  - Causal mask on diagonal blocks via gpsimd affine_select (Pool engine
    otherwise idle).
  - O chains: pT block stationary, V(+ones col) streaming 65 cols; the
    ones column yields the softmax denominator in output column 64.
    Normalization: batched reciprocal over 4 chains + one broadcast
    tensor_mul; output written bf16 (halves the output DMA).
  - All input DMAs issued upfront (4 resident slice buffers).
"""

import numpy as np
import ml_dtypes
from contextlib import ExitStack

import concourse.bass as bass
import concourse.mybir as mybir
import concourse.tile as tile
from concourse import bacc
from concourse.bass_utils import run_bass_kernel_spmd

B, L, H, E = 2, 2048, 16, 64
N_CORES = 8
NS = (B * H) // N_CORES  # slices per core = 4
NT = L // 128  # 16 tiles along l and m
NBLK = NT * (NT + 1) // 2  # 136 causal blocks per slice
SCALE = 0.0625  # 1/sqrt(E) / 2 (K-dup S matmul computes 2*S)
LOG2E = 1.4426950408889634
SCH_A = SCALE * LOG2E * 128.0  # int16 bits = x*SCH_A + SCH_B ~ bf16(exp(x*SCALE))
SCH_B = 128.0 * (127.0 - 0.043)
F32 = mybir.dt.float32
BF16 = mybir.dt.bfloat16
I16 = mybir.dt.int16
BF16NP = ml_dtypes.bfloat16

RND = 12  # blocks per round
ACT_R = 8  # first 8 -> ScalarE exp (2 banks); last 4 -> DVE schraudolph (1 bank)


def _base(mi):
    return NT * mi - (mi * (mi - 1)) // 2


def _plan():
    # global emission order: slice-major, lp-window-major with windows
    # REVERSED (3,2,1,0). All inputs of O-chain li live in window li//4, so
    # each slice's heavy window-3 chain burst (58 MMs) fires mid-slice with
    # S-fills to overlap, and the kernel tail ends on window 0 (10 MMs).
    blocks = []
    for s in range(NS):
        for w in (3, 2, 1, 0):
            for mi in range(4 * w + 4):
                for li in range(max(mi, 4 * w), 4 * w + 4):
                    blocks.append((s, mi, li))
    G = len(blocks)
    n_r = (G + RND - 1) // RND

    # S matmul runs: consecutive blocks, same slice/mi, within one psum
    # bank of one round (<= 4 blocks = 512 cols)
    runs = []
    cur = None
    for g, (s, mi, li) in enumerate(blocks):
        key = (s, mi, g // RND, (g % RND) // 4)
        if cur is not None and cur["key"] == key:
            cur["n"] += 1
        else:
            cur = {"key": key, "s": s, "mi": mi, "li0": li,
                   "j0": g % RND, "g0": g, "n": 1}
            runs.append(cur)

    # consumer segments per round, split at the ACT/DVE boundary and at
    # slice boundaries (different pT destination tiles)
    segs = []
    for r in range(n_r):
        g0, g1 = r * RND, min(G, (r + 1) * RND)
        parts = []
        for kind, a, b in (("act", g0, min(g1, g0 + ACT_R)),
                           ("dve", min(g1, g0 + ACT_R), g1)):
            x = a
            while x < b:
                s = blocks[x][0]
                y = min(b, NBLK * (x // NBLK + 1))
                parts.append((kind, s, x, y))
                x = y
        segs.append(parts)

    # pT column index of each block within its slice (emission order)
    ecol = {}
    for g, (s, mi, li) in enumerate(blocks):
        ecol[(s, mi, li)] = g - NBLK * s

    # O-chain triggers: chain (s, li) becomes ready after the consumers
    # of the round containing its diagonal block (mi=li, li) -- the last
    # emitted of its inputs
    trig = {}
    for s in range(NS):
        for li in range(NT):
            gd = NBLK * s + ecol[(s, li, li)]
            trig.setdefault(gd // RND, []).append((gd, s, li))
    # within a round, order chains by emission position of their diagonal
    # block so windows never interleave (the shared po tile switches on
    # each window's first chain)
    trig = {r: [(s, li) for _, s, li in sorted(v)] for r, v in trig.items()}
    return blocks, runs, segs, trig, ecol, n_r


BLOCKS, RUNS, SEGS, TRIG, ECOL, N_R = _plan()


def _build():
    nc = bacc.Bacc(
        "TRN2",
        target_bir_lowering=False,
        debug=False,
        enable_asserts=True,
        num_devices=N_CORES,
    )
    qT = nc.dram_tensor("qT", [NS, E, L], BF16, kind="ExternalInput").ap()
    kT = nc.dram_tensor("kT", [NS, E, L], BF16, kind="ExternalInput").ap()
    v = nc.dram_tensor("v", [NS, L, E], BF16, kind="ExternalInput").ap()
    outT = nc.dram_tensor("outT", [NS, L, E], BF16, kind="ExternalOutput").ap()

    with tile.TileContext(nc) as tc:
        with ExitStack() as ctx:

            def pool(name, bufs, space="SBUF"):
                return ctx.enter_context(
                    tc.tile_pool(name=name, bufs=bufs, space=space)
                )

            io_q = pool("io_q", NS)
            io_k = pool("io_k", NS)
            vp = pool("vp", 1)
            ptp = pool("pt", 2)
            rp = pool("rp", 2)
            op = pool("op", 2)
            ps_ap = pool("ps_a", 2, "PSUM")
            ps_dp = pool("ps_d", 2, "PSUM")
            ps_op = pool("ps_o", 2, "PSUM")

            # all input DMAs upfront; 4 resident slice buffers
            qts, kts = [], []
            for s in range(NS):
                q_sb = io_q.tile([128, L], BF16, name="q_sb", tag="q")
                nc.sync.dma_start(q_sb[0:E, :], qT[s])
                nc.sync.dma_start(q_sb[E:128, :], qT[s])
                k_sb = io_k.tile([128, L], BF16, name="k_sb", tag="k")
                nc.sync.dma_start(k_sb[0:E, :], kT[s])
                nc.sync.dma_start(k_sb[E:128, :], kT[s])
                qts.append(q_sb)
                kts.append(k_sb)
            # V tiles [128, 65] per (slice, mi): col 64 stays 1.0 forever
            # (denominator trick)
            v_all = vp.tile([128, NS * NT * 65], BF16, name="v_all")
            v4 = v_all.rearrange("p (s t x) -> p s t x", s=NS, t=NT, x=65)
            nc.gpsimd.memset(v4[:, :, :, 64:65], 1.0)
            for s in range(NS):
                nc.sync.dma_start(
                    v4[:, s, :, 0:E],
                    v[s].rearrange("(t p) e -> p t e", p=128),
                )

            # PE warmup: the HAM clock gate needs ~3.4us of sustained matmul
            # activity before the PE runs at 2.4 GHz (cold = 1.2 GHz). Burn
            # dummy matmuls into a scratch PSUM bank while the input DMAs
            # stream so the first real S matmuls run warm.
            wu_sb = vp.tile([128, 128], BF16, name="wu_sb")
            nc.gpsimd.memset(wu_sb[:, :], 0.0)
            wu_ps = ps_op.tile([128, 128], F32, name="wu_ps", tag="po")
            for _ in range(44):
                nc.tensor.matmul(
                    wu_ps[:, :], lhsT=wu_sb[:, :], rhs=wu_sb[:, :],
                    start=True, stop=True, skip_group_check=True,
                )

            pts = {}

            def pt_of(s):
                if s not in pts:
                    pts[s] = ptp.tile(
                        [128, NBLK * 128], BF16, name="ptile", tag="pt"
                    )
                return pts[s]

            state = {"po": None}

            def chain_mm(s, li, mi):
                w, c = li // 4, li % 4
                if c == 0 and mi == 0:
                    state["po"] = ps_op.tile([128, 4 * 65], F32, name="po",
                                             tag="po")
                po = state["po"]
                pT = pt_of(s)
                e = ECOL[(s, mi, li)]
                nc.tensor.matmul(
                    po[:, 65 * c : 65 * c + 65],
                    lhsT=pT[:, 128 * e : 128 * e + 128],
                    rhs=v4[:, s, mi, :],
                    start=(mi == 0),
                    stop=(mi == li),
                    skip_group_check=True,
                )
                if c == 3 and mi == li:
                    po4 = po.rearrange("p (c x) -> p c x", c=4, x=65)
                    r_sb = rp.tile([128, 4], F32, name="r_sb", tag="r")
                    nc.vector.reciprocal(r_sb[:, :], po4[:, :, 64:65])
                    o_sb = op.tile([128, 4 * E], BF16, name="o_sb", tag="o")
                    nc.vector.tensor_mul(
                        o_sb.rearrange("p (c e) -> p c e", c=4, e=E),
                        po4[:, :, 0:E],
                        r_sb.unsqueeze(2).broadcast_to([128, 4, E]),
                    )
                    dst = outT[s].rearrange("(w c p) e -> w p c e", c=4,
                                            p=128)[w]
                    nc.sync.dma_start(
                        dst, o_sb.rearrange("p (c e) -> p c e", c=4, e=E)
                    )

            cq = []

            def drain(n):
                for _ in range(min(n, len(cq))):
                    chain_mm(*cq.pop(0))

            ri = 0
            G = len(BLOCKS)
            pending = []
            cur_ps = {}
            for r in range(N_R):
                g_end = min(G, (r + 1) * RND)
                # S matmuls of round r (allocate this round's psum tiles)
                pa = ps_ap.tile([128, 2 * 512], F32, name="pa", tag="pa")
                pd = None
                while ri < len(RUNS) and RUNS[ri]["g0"] < g_end:
                    rn = RUNS[ri]
                    j0 = rn["j0"]
                    if j0 < ACT_R:
                        dst = pa[:, 128 * j0 : 128 * (j0 + rn["n"])]
                    else:
                        if pd is None:
                            pd = ps_dp.tile([128, 512], F32, name="pd",
                                            tag="pd")
                        dst = pd[:, 128 * (j0 - ACT_R) : 128 * (j0 - ACT_R + rn["n"])]
                    nc.tensor.matmul(
                        dst,
                        lhsT=kts[rn["s"]][:, 128 * rn["mi"] : 128 * rn["mi"] + 128],
                        rhs=qts[rn["s"]][:, 128 * rn["li0"] : 128 * (rn["li0"] + rn["n"])],
                        start=True,
                        stop=True,
                    )
                    ri += 1
                    drain(rn["n"])
                # consumers of round r
                for kind, s, a, b in SEGS[r]:
                    el0 = a - NBLK * s
                    pT = pt_of(s)
                    dst = pT[:, 128 * el0 : 128 * (el0 + (b - a))]
                    if kind == "act":
                        ja = a - r * RND
                        src = pa[:, 128 * ja : 128 * (ja + (b - a))]
                        nc.scalar.activation(
                            dst, src, mybir.ActivationFunctionType.Exp,
                            scale=SCALE,
                        )
                    else:
                        ja = a - r * RND - ACT_R
                        src = pd[:, 128 * ja : 128 * (ja + (b - a))]
                        nc.vector.tensor_scalar(
                            dst.bitcast(I16), src, SCH_A, SCH_B,
                            mybir.AluOpType.mult, mybir.AluOpType.add,
                        )
                # causal mask on diagonal blocks just exp'd
                for (s, li) in TRIG.get(r, []):
                    pT = pt_of(s)
                    e = ECOL[(s, li, li)]
                    seg = pT[:, 128 * e : 128 * e + 128]
                    nc.gpsimd.affine_select(
                        out=seg,
                        in_=seg,
                        pattern=[[1, 128]],
                        compare_op=mybir.AluOpType.is_ge,
                        fill=0.0,
                        base=0,
                        channel_multiplier=-1,
                    )
                for (s, li) in TRIG.get(r, []):
                    for mi in range(li + 1):
                        cq.append((s, li, mi))
            drain(len(cq))

    nc.compile()
    return nc


_NC_CACHE = {}


def _get_nc():
    if "nc" not in _NC_CACHE:
        _NC_CACHE["nc"] = _build()
    return _NC_CACHE["nc"]


def kernel(queries, keys, values, trace=False, tmpdir=None):
    nc = _get_nc()

    # shard: slice g = b*H + h; per-core slices [4c, 4c+4)
    qTf = np.ascontiguousarray(
        queries.transpose(0, 2, 3, 1).reshape(B * H, E, L)
    ).astype(BF16NP)
    kTf = np.ascontiguousarray(
        keys.transpose(0, 2, 3, 1).reshape(B * H, E, L)
    ).astype(BF16NP)
    vf = np.ascontiguousarray(
        values.transpose(0, 2, 1, 3).reshape(B * H, L, E)
    ).astype(BF16NP)

    in_maps = [
        {
            "qT": qTf[NS * c : NS * (c + 1)],
            "kT": kTf[NS * c : NS * (c + 1)],
            "v": vf[NS * c : NS * (c + 1)],
        }
        for c in range(N_CORES)
    ]

    res = run_bass_kernel_spmd(
        nc, in_maps, core_ids=list(range(N_CORES)), trace=trace, tmpdir=tmpdir
    )

    outT = np.concatenate(
        [np.asarray(res.results[c]["outT"]) for c in range(N_CORES)], axis=0
    )
    # outT: [B*H, L, E] bf16 -> [B, L, H, E] f32
    out = outT.astype(np.float32).reshape(B, H, L, E).transpose(0, 2, 1, 3)
    out = np.ascontiguousarray(out, dtype=np.float32)
    if trace:
        kernel.last_exec_time_ns = res.exec_time_ns
    return out
